# revision 30
# baseline (speedup 1.0000x reference)
"""Causal multi-head attention on 8 TRN2 NeuronCores.

Sharding: tensor-parallel over heads. Each core owns 2 of the 16 heads:
column slices of Wq/Wk/Wv. The output projection is fully local: after
attention, small half-batch AllToAlls (512KB fp16 each) redistribute
ctx^T so every core holds ALL 1024 features for its 128-token shard of
each half-batch, then out = ctx @ Wo + bo locally -- no reduction
collective at all (vs. row-parallel Wo + ReduceScatter, this moves 8x
less data). A tiny warmup AllToAll is issued first so the one-time cc
channel-setup barrier overlaps stage A instead of delaying the first
real collective.

Shapes (hardcoded): B=2, S=2048, D=1024, H=16, HD=64.

Numerics: all-fp16 operands, fp32 PSUM accumulation and softmax
denominators (fp8 was tested and rejected: a random-sign dot product
keeps per-element relative error, so fp8 anywhere on the Q/K/V/x path
costs 1e-2..5e-2 of output error).

x is pre-transposed on the host (free) and DMA'd linearly -- no
on-device transpose at all. Stage-A phases are sized progressively
(512, 512, 1024, 1024, 1024 tokens) so the PE starts projecting ~5us
in while the rest of x^T streams. DMA queue roles are fixed: x^T loads
on sync+scalar, dependency-gated DMAs (a2a_in, gather, out) on gpsimd
so a waiting DMA never head-of-line-blocks a prefetch.

Per-core dataflow:
  A) per phase: QT/KT = W_c.T @ xT (8-step K=1024 accumulation,
     N=512); V in natural [tok, feat] layout via lhsT=xT tiles, stored
     per (batch, k-tile, head) as [128, 65] = [V_head | ones-column].
  B) per (batch, 512-query-chunk, head): scores^T[k,q] = KT.T @ QT
     (K=64), exp on ACT straight out of PSUM (scale=0.125, no max
     subtraction: |scores|/8 <~ 3) into paired 2-bank tiles -> fp16,
     causal zeroing of diagonal blocks via gpsimd affine_select on the
     exp output (keep col >= row), then ctx^T[d,q] accumulated over
     k-tiles with lhsT=[V|1] so PSUM row 64 is the softmax denominator.
     Normalize via DVE reciprocal_approx_fast (5x faster than the
     table-based reciprocal, 18-bit accuracy is plenty for softmax
     denominators; input staged through SBUF -- the custom-DVE op
     NaNs on PSUM reads) + gpsimd partition-broadcast + DVE multiply
     in 128-column pieces (lets each a2a_in chunk DMA fly as soon as
     its columns are normalized).
  C) per half-batch: 8 DMAs push ctx^T [128, 128]-token chunks to
     DRAM, AllToAll redistributes, gather to SBUF, local out-proj
     (K=1024 vs full Wo), bias via DVE scalar_tensor_tensor, DMA out.
     Out-projections are emitted late in the PE stream so a straggling
     collective (cross-core start skew is 20-50us) never stalls the PE
     mid-kernel; Wo's 2MB load is deferred past the x^T prefetch window.

Measured: 250-269us HW exec (baseline 423us), rel err 5.4e-4.
"""

import numpy as np

import concourse.bacc as bacc
import concourse.bass as bass
import concourse.mybir as mybir
from concourse.bass_utils import run_bass_kernel_spmd
from concourse.tile import TileContext

B, S, D, H = 2, 2048, 1024, 16
HD = D // H            # 64
NCORES = 8
HPC = H // NCORES      # 2 heads per core
FPC = HPC * HD         # 128 feature cols per core
T = B * S              # 4096 tokens
SPAN = 512             # stage-A token span
NSPAN = T // SPAN      # 8
QC = 512               # query chunk
NCHB = S // QC         # 4 chunks per batch
KT = 128               # k-tile size
HDP = HD + 1           # [V|1] tile width
NKT = S // KT          # 16 k-tiles per batch
TPB = S // NCORES      # 256 tokens per core per batch
TPH = TPB // 2         # 128 tokens per core per half-batch (a2a chunk)
F32 = mybir.dt.float32
F16 = mybir.dt.float16
F8 = mybir.dt.float8e4
DR = mybir.MatmulPerfMode.DoubleRow
WSCALE = 1.0
EXP_SCALE = 0.125 / (WSCALE * WSCALE)
OUT_SCALE = 1.0


def build_nc():
    nc = bacc.Bacc(num_devices=NCORES)

    xt_d = nc.dram_tensor("xt", [D, T], F16, kind="ExternalInput")
    wq_d = nc.dram_tensor("wq", [D, FPC], F16, kind="ExternalInput")
    wk_d = nc.dram_tensor("wk", [D, FPC], F16, kind="ExternalInput")
    wv_d = nc.dram_tensor("wv", [D, FPC], F16, kind="ExternalInput")
    wo_d = nc.dram_tensor("wo", [D, D], F16, kind="ExternalInput")
    bo_d = nc.dram_tensor("bo", [1, D], F32, kind="ExternalInput")
    warm_in = nc.dram_tensor("warm_in", [NCORES, 16], F32, kind="Internal")
    warm_out = nc.dram_tensor("warm_out", [NCORES, 16], F32, kind="Internal")
    a2a_in = [nc.dram_tensor(f"a2a_in{i}", [D, TPH], F16, kind="Internal")
              for i in range(2 * B)]
    a2a_out = [nc.dram_tensor(f"a2a_out{i}", [D, TPH], F16, kind="Internal")
               for i in range(2 * B)]
    out_d = nc.dram_tensor("out", [B * TPB, D], F16, kind="ExternalOutput")

    groups = [list(range(NCORES))]

    with TileContext(nc) as tc:
        with (
            tc.tile_pool(name="const", bufs=1) as constp,
            tc.tile_pool(name="wts", bufs=1) as wp,
            tc.tile_pool(name="big", bufs=1) as bigp,
        ):
            # warmup collective first: absorbs the cc-channel setup
            # barrier while stage A runs.
            nc.gpsimd.collective_compute(
                "AllToAll", mybir.AluOpType.bypass, replica_groups=groups,
                ins=[warm_in[:, :]], outs=[warm_out[:, :]],
            )

            # --- weights / constants ---
            wq_sb = wp.tile([128, 8, FPC], F16)
            wk_sb = wp.tile([128, 8, FPC], F16)
            wv_sb = wp.tile([128, 8, FPC], F16)
            for w_sb, w_dram in ((wq_sb, wq_d), (wk_sb, wk_d), (wv_sb, wv_d)):
                for j in range(8):
                    nc.scalar.dma_start(w_sb[:, j, :], w_dram[j * 128:(j + 1) * 128, :])
            wo_sb = wp.tile([128, 8, D], F16)
            bo_row = constp.tile([1, D], F32)
            nc.scalar.dma_start(bo_row, bo_d[0:1, :])
            bo_bc = constp.tile([128, D], F32)
            nc.gpsimd.partition_broadcast(bo_bc, bo_row)

            # --- resident activations ---
            qt_sb = bigp.tile([128, T], F16)     # Q^T  [feat(2 heads x 64), tok]
            kt_sb = bigp.tile([128, T], F16)     # K^T
            ctxt = bigp.tile([128, T], F16)      # normalized ctx^T (fp16)
            v16 = bigp.tile([128, B, NKT, HPC, HDP], F16)  # [V_h|1] tiles
            ones_col = constp.tile([128, 1], F32)
            nc.gpsimd.memset(ones_col, 1.0)
            nc.vector.tensor_copy(
                v16[:, :, :, :, HD:HD + 1],
                ones_col[:, None, None, None, :].broadcast_to([128, B, NKT, HPC, 1]),
            )

            with (
                tc.tile_pool(name="xt", bufs=3) as xtp,
                tc.tile_pool(name="ex", bufs=3) as sbB,
                tc.tile_pool(name="nrm", bufs=2) as nrm,
                tc.tile_pool(name="ga", bufs=2) as gap,
                tc.tile_pool(name="sbO", bufs=2) as sbO,
                tc.tile_pool(name="psA", bufs=2, space="PSUM") as psA,
                tc.tile_pool(name="psS", bufs=2, space="PSUM") as psS,
                tc.tile_pool(name="psC", bufs=2, space="PSUM") as psC,
            ):
                APH = 2 * SPAN      # max stage-A phase width
                PHASES = [(0, 512), (512, 512), (1024, 1024),
                          (2048, 1024), (3072, 1024)]

                def emit_a_dma(ph, three_q=False):
                    t0, w = PHASES[ph]
                    xt = xtp.tile([128, 8, APH], F16, tag="xt")
                    for j in range(8):
                        if three_q:
                            eng = (nc.sync, nc.scalar, nc.gpsimd)[j % 3]
                        else:
                            eng = nc.sync if j % 2 == 0 else nc.scalar
                        eng.dma_start(
                            xt[:, j, :w],
                            xt_d[j * 128:(j + 1) * 128, t0:t0 + w])
                    return xt

                def emit_a_proj(xt, ph):
                    t0, w = PHASES[ph]
                    b = t0 // S
                    for hv in range(w // SPAN):
                        for w_sb, dst in ((wq_sb, qt_sb), (wk_sb, kt_sb)):
                            pp = psA.tile([128, SPAN], F32, tag="p")
                            for j in range(8):
                                nc.tensor.matmul(
                                    pp, w_sb[:, j, :],
                                    xt[:, j, hv * SPAN:(hv + 1) * SPAN],
                                    start=(j == 0), stop=(j == 7),
                                )
                            nc.vector.tensor_copy(
                                dst[:, t0 + hv * SPAN:t0 + (hv + 1) * SPAN], pp)
                    for t in range(w // 128):
                        kti = (t0 + t * 128 - b * S) // KT
                        pv = psA.tile([128, SPAN], F32, tag="p")
                        for j in range(8):
                            nc.tensor.matmul(
                                pv[:, 0:FPC],
                                xt[:, j, t * 128:(t + 1) * 128],
                                wv_sb[:, j, :],
                                start=(j == 0), stop=(j == 7),
                            )
                        for h in range(HPC):
                            nc.vector.tensor_copy(
                                v16[:, b, kti, h, 0:HD],
                                pv[:, h * HD:(h + 1) * HD],
                            )

                def emit_a(ph):
                    emit_a_proj(emit_a_dma(ph), ph)

                def emit_attn(b, qc):
                    q0 = b * S + qc * QC
                    for h in range(HPC):
                        pc = psC.tile([HDP, QC], F32, tag="c")
                        n_full = qc * 4
                        # full k-tiles below the diagonal, in DoubleRow pairs
                        for p in range(n_full // 2):
                            kt0 = 2 * p
                            ps = psS.tile([128, 2, QC], F32, tag="s")
                            for i in range(2):
                                nc.tensor.matmul(
                                    ps[:, i, :],
                                    kt_sb[h * HD:(h + 1) * HD,
                                          b * S + (kt0 + i) * KT:
                                          b * S + (kt0 + i + 1) * KT],
                                    qt_sb[h * HD:(h + 1) * HD, q0:q0 + QC],
                                    start=True, stop=True,
                                )
                            ex = sbB.tile([128, 2, QC], F16, tag="ex")
                            nc.scalar.activation(
                                ex, ps, mybir.ActivationFunctionType.Exp,
                                scale=EXP_SCALE,
                            )
                            for i in range(2):
                                nc.tensor.matmul(
                                    pc[:, :],
                                    v16[:, b, kt0 + i, h, :], ex[:, i, :],
                                    start=(p == 0 and i == 0), stop=False,
                                )
                        # 4 diagonal k-tiles, singles with causal zeroing
                        for dgi in range(4):
                            kt = qc * 4 + dgi
                            col_off = dgi * KT
                            n = QC - col_off
                            ps1 = psS.tile([128, 2, QC], F32, tag="s")
                            nc.tensor.matmul(
                                ps1[:, 0, :n],
                                kt_sb[h * HD:(h + 1) * HD,
                                      b * S + kt * KT:b * S + (kt + 1) * KT],
                                qt_sb[h * HD:(h + 1) * HD, q0 + col_off:q0 + QC],
                                start=True, stop=True,
                            )
                            ex1 = sbB.tile([128, 2, QC], F16, tag="ex")
                            nc.scalar.activation(
                                ex1[:, 0, :n], ps1[:, 0, :n],
                                mybir.ActivationFunctionType.Exp,
                                scale=EXP_SCALE,
                            )
                            nc.gpsimd.affine_select(
                                out=ex1[:, 0, 0:KT],
                                in_=ex1[:, 0, 0:KT],
                                compare_op=mybir.AluOpType.is_ge,
                                fill=0.0,
                                base=0,
                                pattern=[[1, KT]],
                                channel_multiplier=-1,
                            )
                            nc.tensor.matmul(
                                pc[:, col_off:QC],
                                v16[:, b, kt, h, :],
                                ex1[:, 0, :n],
                                start=(n_full == 0 and dgi == 0),
                                stop=(dgi == 3),
                            )
                        den = nrm.tile([1, QC], F32, tag="d")
                        nc.vector.tensor_copy(den, pc[HD:HD + 1, :])
                        rrow = nrm.tile([1, QC], F32, tag="r")
                        nc.vector.reciprocal_approx_fast(rrow, den)
                        rec64 = nrm.tile([HD, QC], F32, tag="b")
                        nc.gpsimd.partition_broadcast(rec64, rrow)
                        for pz in range(4):
                            z = pz * 128
                            nc.vector.tensor_mul(
                                ctxt[h * HD:(h + 1) * HD, q0 + z:q0 + z + 128],
                                pc[0:HD, z:z + 128], rec64[:, z:z + 128],
                            )

                def emit_a2a(b, hf):
                    i = 2 * b + hf
                    c0 = b * S + hf * (S // 2)
                    dma_eng = nc.scalar if i == 2 * B - 1 else nc.gpsimd
                    for d in range(NCORES):
                        dma_eng.dma_start(
                            a2a_in[i][d * 128:(d + 1) * 128, :],
                            ctxt[:, c0 + d * TPH:c0 + (d + 1) * TPH])
                    nc.gpsimd.collective_compute(
                        "AllToAll", mybir.AluOpType.bypass,
                        replica_groups=groups,
                        ins=[a2a_in[i][:, :]], outs=[a2a_out[i][:, :]],
                    )

                def emit_out(b, hf):
                    i = 2 * b + hf
                    ga = gap.tile([128, 8, TPH], F16, tag="ga")
                    for c in range(NCORES):
                        nc.sync.dma_start(
                            ga[:, c, :], a2a_out[i][c * 128:(c + 1) * 128, :])
                    so = sbO.tile([128, D], F16, tag="so")
                    for half in range(2):
                        po = psA.tile([128, SPAN], F32, tag="p")
                        for j in range(8):
                            nc.tensor.matmul(
                                po,
                                ga[:, j, :],
                                wo_sb[:, j, half * 512:(half + 1) * 512],
                                start=(j == 0), stop=(j == 7),
                            )
                        nc.vector.scalar_tensor_tensor(
                            so[:, half * 512:(half + 1) * 512],
                            po, OUT_SCALE, bo_bc[:, half * 512:(half + 1) * 512],
                            mybir.AluOpType.mult, mybir.AluOpType.add,
                        )
                    nc.gpsimd.dma_start(
                        out_d[b * TPB + hf * TPH:b * TPB + (hf + 1) * TPH, :], so)

                xts012 = [emit_a_dma(ph, three_q=True) for ph in range(3)]
                emit_a_proj(xts012[0], 0)
                emit_attn(0, 0)
                emit_a_proj(xts012[1], 1)
                emit_attn(0, 1)
                emit_a2a(0, 0)
                emit_a_proj(xts012[2], 2)
                emit_attn(0, 2)
                emit_attn(0, 3)
                emit_a2a(0, 1)
                emit_a(3)
                for j in range(8):
                    nc.scalar.dma_start(
                        wo_sb[:, j, :], wo_d[j * 128:(j + 1) * 128, :])
                emit_attn(1, 0)
                emit_attn(1, 1)
                emit_a(4)
                emit_a2a(1, 0)
                emit_out(0, 0)
                emit_attn(1, 2)
                emit_out(0, 1)
                emit_attn(1, 3)
                emit_out(1, 0)
                emit_a2a(1, 1)
                emit_out(1, 1)

    nc.finalize()
    return nc


_NC_CACHE = []


def make_in_maps(x, Wq, Wk, Wv, Wo, bo):
    x = np.asarray(x, dtype=np.float32).reshape(T, D)
    xt16 = np.ascontiguousarray(x.T).astype(np.float16)
    Wq = np.asarray(Wq, dtype=np.float32)
    Wk = np.asarray(Wk, dtype=np.float32)
    Wv = np.asarray(Wv, dtype=np.float32)
    wo16 = np.asarray(Wo, dtype=np.float32).astype(np.float16)
    bo = np.asarray(bo, dtype=np.float32).reshape(1, D)
    in_maps = []
    for c in range(NCORES):
        lo, hi = c * FPC, (c + 1) * FPC
        in_maps.append({
            "xt": xt16,
            "wq": np.ascontiguousarray(Wq[:, lo:hi]).astype(np.float16),
            "wk": np.ascontiguousarray(Wk[:, lo:hi]).astype(np.float16),
            "wv": np.ascontiguousarray(Wv[:, lo:hi]).astype(np.float16),
            "wo": wo16,
            "bo": bo,
        })
    return in_maps


def assemble_out(core_outs):
    # core r rows [b*256 + hf*128 + i] = batch b, s = hf*1024 + r*128 + i
    full = np.empty((B, S, D), dtype=np.float32)
    for r, o in enumerate(core_outs):
        o = np.asarray(o, dtype=np.float32)
        for b in range(B):
            for hf in range(2):
                full[b, hf * (S // 2) + r * TPH:hf * (S // 2) + (r + 1) * TPH] = \
                    o[b * TPB + hf * TPH:b * TPB + (hf + 1) * TPH]
    return full


def kernel(x, Wq, Wk, Wv, Wo, bo):
    if not _NC_CACHE:
        _NC_CACHE.append(build_nc())
    nc = _NC_CACHE[0]
    in_maps = make_in_maps(x, Wq, Wk, Wv, Wo, bo)
    res = run_bass_kernel_spmd(nc, in_maps, core_ids=list(range(NCORES)))
    return assemble_out([r["out"] for r in res.results])


# revision 31
# speedup vs baseline: 1.0797x; 1.0797x over previous
"""Causal multi-head attention on 8 TRN2 NeuronCores.

Sharding: tensor-parallel over heads. Each core owns 2 of the 16 heads:
column slices of Wq/Wk/Wv. The output projection is fully local: after
attention, small half-batch AllToAlls (512KB fp16 each) redistribute
ctx^T so every core holds ALL 1024 features for its 128-token shard of
each half-batch, then out = ctx @ Wo + bo locally -- no reduction
collective at all (vs. row-parallel Wo + ReduceScatter, this moves 8x
less data). A tiny warmup AllToAll is issued first so the one-time cc
channel-setup barrier overlaps stage A instead of delaying the first
real collective.

Shapes (hardcoded): B=2, S=2048, D=1024, H=16, HD=64.

Numerics: all-fp16 operands, fp32 PSUM accumulation and softmax
denominators (fp8 was tested and rejected: a random-sign dot product
keeps per-element relative error, so fp8 anywhere on the Q/K/V/x path
costs 1e-2..5e-2 of output error).

x is pre-transposed on the host (free) and DMA'd linearly -- no
on-device transpose at all. Stage-A phases are sized progressively
(512, 512, 1024, 1024, 1024 tokens) so the PE starts projecting ~5us
in while the rest of x^T streams. DMA queue roles are fixed: x^T loads
on sync+scalar, dependency-gated DMAs (a2a_in, gather, out) on gpsimd
so a waiting DMA never head-of-line-blocks a prefetch.

Per-core dataflow:
  A) per phase: QT/KT = W_c.T @ xT (8-step K=1024 accumulation,
     N=512); V in natural [tok, feat] layout via lhsT=xT tiles, stored
     per (batch, k-tile, head) as [128, 65] = [V_head | ones-column].
  B) per (batch, 512-query-chunk, head): scores^T[k,q] = KT.T @ QT
     (K=64), exp on ACT straight out of PSUM (scale=0.125, no max
     subtraction: |scores|/8 <~ 3) into paired 2-bank tiles -> fp16,
     causal zeroing of diagonal blocks via gpsimd affine_select on the
     exp output (keep col >= row), then ctx^T[d,q] accumulated over
     k-tiles with lhsT=[V|1] so PSUM row 64 is the softmax denominator.
     Normalize via DVE reciprocal_approx_fast (5x faster than the
     table-based reciprocal, 18-bit accuracy is plenty for softmax
     denominators; input staged through SBUF -- the custom-DVE op
     NaNs on PSUM reads) + gpsimd partition-broadcast + DVE multiply
     in 128-column pieces (lets each a2a_in chunk DMA fly as soon as
     its columns are normalized).
  C) per half-batch: 8 DMAs push ctx^T [128, 128]-token chunks to
     DRAM, AllToAll redistributes, gather to SBUF, local out-proj
     (K=1024 vs full Wo), bias via DVE scalar_tensor_tensor, DMA out.
     Out-projections are emitted late in the PE stream so a straggling
     collective (cross-core start skew is 20-50us) never stalls the PE
     mid-kernel; Wo's 2MB load is deferred past the x^T prefetch window.

Measured: 250-269us HW exec (baseline 423us), rel err 5.4e-4.
"""

import numpy as np

import concourse.bacc as bacc
import concourse.bass as bass
import concourse.mybir as mybir
from concourse.bass_utils import run_bass_kernel_spmd
from concourse.tile import TileContext

B, S, D, H = 2, 2048, 1024, 16
HD = D // H            # 64
NCORES = 8
HPC = H // NCORES      # 2 heads per core
FPC = HPC * HD         # 128 feature cols per core
T = B * S              # 4096 tokens
SPAN = 512             # stage-A token span
NSPAN = T // SPAN      # 8
QC = 512               # query chunk
NCHB = S // QC         # 4 chunks per batch
KT = 128               # k-tile size
HDP = HD + 1           # [V|1] tile width
NKT = S // KT          # 16 k-tiles per batch
TPB = S // NCORES      # 256 tokens per core per batch
TPH = TPB // 2         # 128 tokens per core per half-batch (a2a chunk)
F32 = mybir.dt.float32
F16 = mybir.dt.float16
F8 = mybir.dt.float8e4
DR = mybir.MatmulPerfMode.DoubleRow
WSCALE = 1.0
EXP_SCALE = 0.125 / (WSCALE * WSCALE)
OUT_SCALE = 1.0


def build_nc():
    nc = bacc.Bacc(num_devices=NCORES)

    xt_d = nc.dram_tensor("xt", [D, T], F16, kind="ExternalInput")
    wq_d = nc.dram_tensor("wq", [D, FPC], F16, kind="ExternalInput")
    wk_d = nc.dram_tensor("wk", [D, FPC], F16, kind="ExternalInput")
    wv_d = nc.dram_tensor("wv", [D, FPC], F16, kind="ExternalInput")
    wo_d = nc.dram_tensor("wo", [D, D], F16, kind="ExternalInput")
    bo_d = nc.dram_tensor("bo", [1, D], F32, kind="ExternalInput")
    warm_in = nc.dram_tensor("warm_in", [NCORES, 16], F32, kind="Internal")
    warm_out = nc.dram_tensor("warm_out", [NCORES, 16], F32, kind="Internal")
    a2a_in = [nc.dram_tensor(f"a2a_in{i}", [D, TPH], F16, kind="Internal")
              for i in range(2 * B)]
    a2a_out = [nc.dram_tensor(f"a2a_out{i}", [D, TPH], F16, kind="Internal")
               for i in range(2 * B)]
    out_d = nc.dram_tensor("out", [B * TPB, D], F16, kind="ExternalOutput")

    groups = [list(range(NCORES))]

    with TileContext(nc) as tc:
        with (
            tc.tile_pool(name="const", bufs=1) as constp,
            tc.tile_pool(name="wts", bufs=1) as wp,
            tc.tile_pool(name="big", bufs=1) as bigp,
        ):
            # warmup collective first: absorbs the cc-channel setup
            # barrier while stage A runs.
            nc.gpsimd.collective_compute(
                "AllToAll", mybir.AluOpType.bypass, replica_groups=groups,
                ins=[warm_in[:, :]], outs=[warm_out[:, :]],
            )

            # --- weights / constants ---
            wq_sb = wp.tile([128, 8, FPC], F16)
            wk_sb = wp.tile([128, 8, FPC], F16)
            wv_sb = wp.tile([128, 8, FPC], F16)
            for w_sb, w_dram in ((wq_sb, wq_d), (wk_sb, wk_d), (wv_sb, wv_d)):
                for j in range(8):
                    nc.scalar.dma_start(w_sb[:, j, :], w_dram[j * 128:(j + 1) * 128, :])
            wo_sb = wp.tile([128, 8, D], F16)
            bo_row = constp.tile([1, D], F32)
            nc.scalar.dma_start(bo_row, bo_d[0:1, :])
            bo_bc = constp.tile([128, D], F32)
            nc.gpsimd.partition_broadcast(bo_bc, bo_row)

            # --- resident activations ---
            qt_sb = bigp.tile([128, T], F16)     # Q^T  [feat(2 heads x 64), tok]
            kt_sb = bigp.tile([128, T], F16)     # K^T
            ctxt = bigp.tile([128, T], F16)      # normalized ctx^T (fp16)
            v16 = bigp.tile([128, B, NKT, HPC, HDP], F16)  # [V_h|1] tiles
            ones_col = constp.tile([128, 1], F32)
            nc.gpsimd.memset(ones_col, 1.0)
            nc.vector.tensor_copy(
                v16[:, :, :, :, HD:HD + 1],
                ones_col[:, None, None, None, :].broadcast_to([128, B, NKT, HPC, 1]),
            )

            with (
                tc.tile_pool(name="xt", bufs=3) as xtp,
                tc.tile_pool(name="ex", bufs=3) as sbB,
                tc.tile_pool(name="nrm", bufs=2) as nrm,
                tc.tile_pool(name="ga", bufs=2) as gap,
                tc.tile_pool(name="sbO", bufs=2) as sbO,
                tc.tile_pool(name="psA", bufs=2, space="PSUM") as psA,
                tc.tile_pool(name="psS", bufs=2, space="PSUM") as psS,
                tc.tile_pool(name="psC", bufs=2, space="PSUM") as psC,
            ):
                APH = 2 * SPAN      # max stage-A phase width
                PHASES = [(0, 512), (512, 512), (1024, 1024),
                          (2048, 1024), (3072, 1024)]

                def emit_a_dma(ph, three_q=False):
                    t0, w = PHASES[ph]
                    xt = xtp.tile([128, 8, APH], F16, tag="xt")
                    for j in range(8):
                        if three_q:
                            eng = (nc.sync, nc.scalar, nc.gpsimd)[j % 3]
                        else:
                            eng = nc.sync if j % 2 == 0 else nc.scalar
                        eng.dma_start(
                            xt[:, j, :w],
                            xt_d[j * 128:(j + 1) * 128, t0:t0 + w])
                    return xt

                def emit_a_proj(xt, ph):
                    t0, w = PHASES[ph]
                    b = t0 // S
                    for hv in range(w // SPAN):
                        for w_sb, dst in ((wq_sb, qt_sb), (wk_sb, kt_sb)):
                            pp = psA.tile([128, SPAN], F32, tag="p")
                            for j in range(8):
                                nc.tensor.matmul(
                                    pp, w_sb[:, j, :],
                                    xt[:, j, hv * SPAN:(hv + 1) * SPAN],
                                    start=(j == 0), stop=(j == 7),
                                )
                            nc.vector.tensor_copy(
                                dst[:, t0 + hv * SPAN:t0 + (hv + 1) * SPAN], pp)
                    for t in range(w // 128):
                        kti = (t0 + t * 128 - b * S) // KT
                        pv = psA.tile([128, SPAN], F32, tag="p")
                        for j in range(8):
                            nc.tensor.matmul(
                                pv[:, 0:FPC],
                                xt[:, j, t * 128:(t + 1) * 128],
                                wv_sb[:, j, :],
                                start=(j == 0), stop=(j == 7),
                            )
                        for h in range(HPC):
                            nc.vector.tensor_copy(
                                v16[:, b, kti, h, 0:HD],
                                pv[:, h * HD:(h + 1) * HD],
                            )

                def emit_a(ph):
                    emit_a_proj(emit_a_dma(ph), ph)

                def emit_attn(b, qc):
                    q0 = b * S + qc * QC
                    for h in range(HPC):
                        pc = psC.tile([HDP, QC], F32, tag="c")
                        n_full = qc * 4
                        # full k-tiles below the diagonal, in DoubleRow pairs
                        for p in range(n_full // 2):
                            kt0 = 2 * p
                            ps = psS.tile([128, 2, QC], F32, tag="s")
                            for i in range(2):
                                nc.tensor.matmul(
                                    ps[:, i, :],
                                    kt_sb[h * HD:(h + 1) * HD,
                                          b * S + (kt0 + i) * KT:
                                          b * S + (kt0 + i + 1) * KT],
                                    qt_sb[h * HD:(h + 1) * HD, q0:q0 + QC],
                                    start=True, stop=True,
                                )
                            ex = sbB.tile([128, 2, QC], F16, tag="ex")
                            nc.scalar.activation(
                                ex, ps, mybir.ActivationFunctionType.Exp,
                                scale=EXP_SCALE,
                            )
                            for i in range(2):
                                nc.tensor.matmul(
                                    pc[:, :],
                                    v16[:, b, kt0 + i, h, :], ex[:, i, :],
                                    start=(p == 0 and i == 0), stop=False,
                                )
                        # 4 diagonal k-tiles, singles with causal zeroing
                        for dgi in range(4):
                            kt = qc * 4 + dgi
                            col_off = dgi * KT
                            n = QC - col_off
                            ps1 = psS.tile([128, 2, QC], F32, tag="s")
                            nc.tensor.matmul(
                                ps1[:, 0, :n],
                                kt_sb[h * HD:(h + 1) * HD,
                                      b * S + kt * KT:b * S + (kt + 1) * KT],
                                qt_sb[h * HD:(h + 1) * HD, q0 + col_off:q0 + QC],
                                start=True, stop=True,
                            )
                            ex1 = sbB.tile([128, 2, QC], F16, tag="ex")
                            nc.scalar.activation(
                                ex1[:, 0, :n], ps1[:, 0, :n],
                                mybir.ActivationFunctionType.Exp,
                                scale=EXP_SCALE,
                            )
                            nc.gpsimd.affine_select(
                                out=ex1[:, 0, 0:KT],
                                in_=ex1[:, 0, 0:KT],
                                compare_op=mybir.AluOpType.is_ge,
                                fill=0.0,
                                base=0,
                                pattern=[[1, KT]],
                                channel_multiplier=-1,
                            )
                            nc.tensor.matmul(
                                pc[:, col_off:QC],
                                v16[:, b, kt, h, :],
                                ex1[:, 0, :n],
                                start=(n_full == 0 and dgi == 0),
                                stop=(dgi == 3),
                            )
                        den = nrm.tile([1, QC], F32, tag="d")
                        nc.vector.tensor_copy(den, pc[HD:HD + 1, :])
                        rrow = nrm.tile([1, QC], F32, tag="r")
                        nc.vector.reciprocal_approx_fast(rrow, den)
                        rec64 = nrm.tile([HD, QC], F32, tag="b")
                        nc.gpsimd.partition_broadcast(rec64, rrow)
                        for pz in range(4):
                            z = pz * 128
                            nc.vector.tensor_mul(
                                ctxt[h * HD:(h + 1) * HD, q0 + z:q0 + z + 128],
                                pc[0:HD, z:z + 128], rec64[:, z:z + 128],
                            )

                def emit_a2a(b, hf):
                    i = 2 * b + hf
                    c0 = b * S + hf * (S // 2)
                    dma_eng = nc.scalar if i == 2 * B - 1 else nc.gpsimd
                    for d in range(NCORES):
                        dma_eng.dma_start(
                            a2a_in[i][d * 128:(d + 1) * 128, :],
                            ctxt[:, c0 + d * TPH:c0 + (d + 1) * TPH])
                    nc.gpsimd.collective_compute(
                        "AllToAll", mybir.AluOpType.bypass,
                        replica_groups=groups,
                        ins=[a2a_in[i][:, :]], outs=[a2a_out[i][:, :]],
                    )

                def emit_out(b, hf):
                    i = 2 * b + hf
                    ga = gap.tile([128, 8, TPH], F16, tag="ga")
                    for c in range(NCORES):
                        nc.sync.dma_start(
                            ga[:, c, :], a2a_out[i][c * 128:(c + 1) * 128, :])
                    so = sbO.tile([128, D], F16, tag="so")
                    for half in range(2):
                        po = psA.tile([128, SPAN], F32, tag="p")
                        for j in range(8):
                            nc.tensor.matmul(
                                po,
                                ga[:, j, :],
                                wo_sb[:, j, half * 512:(half + 1) * 512],
                                start=(j == 0), stop=(j == 7),
                            )
                        nc.vector.scalar_tensor_tensor(
                            so[:, half * 512:(half + 1) * 512],
                            po, OUT_SCALE, bo_bc[:, half * 512:(half + 1) * 512],
                            mybir.AluOpType.mult, mybir.AluOpType.add,
                        )
                    nc.gpsimd.dma_start(
                        out_d[b * TPB + hf * TPH:b * TPB + (hf + 1) * TPH, :], so)

                emit_a(0)
                emit_attn(0, 0)
                emit_a(1)
                emit_attn(0, 1)
                emit_a2a(0, 0)
                emit_a(2)
                emit_attn(0, 2)
                emit_attn(0, 3)
                emit_a2a(0, 1)
                emit_a(3)
                for j in range(8):
                    nc.scalar.dma_start(
                        wo_sb[:, j, :], wo_d[j * 128:(j + 1) * 128, :])
                emit_attn(1, 0)
                emit_attn(1, 1)
                emit_a(4)
                emit_a2a(1, 0)
                emit_out(0, 0)
                emit_attn(1, 2)
                emit_out(0, 1)
                emit_attn(1, 3)
                emit_out(1, 0)
                emit_a2a(1, 1)
                emit_out(1, 1)

    nc.finalize()
    return nc


_NC_CACHE = []


def make_in_maps(x, Wq, Wk, Wv, Wo, bo):
    x = np.asarray(x, dtype=np.float32).reshape(T, D)
    xt16 = np.ascontiguousarray(x.T).astype(np.float16)
    Wq = np.asarray(Wq, dtype=np.float32)
    Wk = np.asarray(Wk, dtype=np.float32)
    Wv = np.asarray(Wv, dtype=np.float32)
    wo16 = np.asarray(Wo, dtype=np.float32).astype(np.float16)
    bo = np.asarray(bo, dtype=np.float32).reshape(1, D)
    in_maps = []
    for c in range(NCORES):
        lo, hi = c * FPC, (c + 1) * FPC
        in_maps.append({
            "xt": xt16,
            "wq": np.ascontiguousarray(Wq[:, lo:hi]).astype(np.float16),
            "wk": np.ascontiguousarray(Wk[:, lo:hi]).astype(np.float16),
            "wv": np.ascontiguousarray(Wv[:, lo:hi]).astype(np.float16),
            "wo": wo16,
            "bo": bo,
        })
    return in_maps


def assemble_out(core_outs):
    # core r rows [b*256 + hf*128 + i] = batch b, s = hf*1024 + r*128 + i
    full = np.empty((B, S, D), dtype=np.float32)
    for r, o in enumerate(core_outs):
        o = np.asarray(o, dtype=np.float32)
        for b in range(B):
            for hf in range(2):
                full[b, hf * (S // 2) + r * TPH:hf * (S // 2) + (r + 1) * TPH] = \
                    o[b * TPB + hf * TPH:b * TPB + (hf + 1) * TPH]
    return full


def kernel(x, Wq, Wk, Wv, Wo, bo):
    if not _NC_CACHE:
        _NC_CACHE.append(build_nc())
    nc = _NC_CACHE[0]
    in_maps = make_in_maps(x, Wq, Wk, Wv, Wo, bo)
    res = run_bass_kernel_spmd(nc, in_maps, core_ids=list(range(NCORES)))
    return assemble_out([r["out"] for r in res.results])


# revision 33
# speedup vs baseline: 1.1396x; 1.0555x over previous
"""Causal multi-head attention on 8 TRN2 NeuronCores.

Sharding: tensor-parallel over heads. Each core owns 2 of the 16 heads:
column slices of Wq/Wk/Wv. The output projection is fully local: after
attention, small half-batch AllToAlls (512KB fp16 each) redistribute
ctx^T so every core holds ALL 1024 features for its 128-token shard of
each half-batch, then out = ctx @ Wo + bo locally -- no reduction
collective at all (vs. row-parallel Wo + ReduceScatter, this moves 8x
less data). A tiny warmup AllToAll is issued first so the one-time cc
channel-setup barrier overlaps stage A instead of delaying the first
real collective.

Shapes (hardcoded): B=2, S=2048, D=1024, H=16, HD=64.

Numerics: all-fp16 operands, fp32 PSUM accumulation and softmax
denominators (fp8 was tested and rejected: a random-sign dot product
keeps per-element relative error, so fp8 anywhere on the Q/K/V/x path
costs 1e-2..5e-2 of output error).

x is pre-transposed on the host (free) and DMA'd linearly -- no
on-device transpose at all. Stage-A phases are sized progressively
(512, 512, 1024, 1024, 1024 tokens) so the PE starts projecting ~5us
in while the rest of x^T streams. DMA queue roles are fixed: x^T loads
on sync+scalar, dependency-gated DMAs (a2a_in, gather, out) on gpsimd
so a waiting DMA never head-of-line-blocks a prefetch.

Per-core dataflow:
  A) per phase: QT/KT = W_c.T @ xT (8-step K=1024 accumulation,
     N=512); V in natural [tok, feat] layout via lhsT=xT tiles, stored
     per (batch, k-tile, head) as [128, 65] = [V_head | ones-column].
  B) per (batch, 512-query-chunk, head): scores^T[k,q] = KT.T @ QT
     (K=64), exp on ACT straight out of PSUM (scale=0.125, no max
     subtraction: |scores|/8 <~ 3) into paired 2-bank tiles -> fp16,
     causal zeroing of diagonal blocks via gpsimd affine_select on the
     exp output (keep col >= row), then ctx^T[d,q] accumulated over
     k-tiles with lhsT=[V|1] so PSUM row 64 is the softmax denominator.
     Normalize via DVE reciprocal_approx_fast (5x faster than the
     table-based reciprocal, 18-bit accuracy is plenty for softmax
     denominators; input staged through SBUF -- the custom-DVE op
     NaNs on PSUM reads) + gpsimd partition-broadcast + DVE multiply
     in 128-column pieces (lets each a2a_in chunk DMA fly as soon as
     its columns are normalized).
  C) per half-batch: 8 DMAs push ctx^T [128, 128]-token chunks to
     DRAM, AllToAll redistributes, gather to SBUF, local out-proj
     (K=1024 vs full Wo), bias via DVE scalar_tensor_tensor, DMA out.
     Out-projections are emitted late in the PE stream so a straggling
     collective (cross-core start skew is 20-50us) never stalls the PE
     mid-kernel; Wo's 2MB load is deferred past the x^T prefetch window.

The final half-batch's a2a_in DMAs issue from the scalar queue
(idle after the last exp) instead of gpsimd, shortening the tail
dispatch chain.

Measured: 250-265us HW exec typical, +/-15us cross-core-skew noise
(baseline 423us); rel err 5.4e-4 (max), 5.7e-4 (l2).
"""

import numpy as np

import concourse.bacc as bacc
import concourse.bass as bass
import concourse.mybir as mybir
from concourse.bass_utils import run_bass_kernel_spmd
from concourse.tile import TileContext

B, S, D, H = 2, 2048, 1024, 16
HD = D // H            # 64
NCORES = 8
HPC = H // NCORES      # 2 heads per core
FPC = HPC * HD         # 128 feature cols per core
T = B * S              # 4096 tokens
SPAN = 512             # stage-A token span
NSPAN = T // SPAN      # 8
QC = 512               # query chunk
NCHB = S // QC         # 4 chunks per batch
KT = 128               # k-tile size
HDP = HD + 1           # [V|1] tile width
NKT = S // KT          # 16 k-tiles per batch
TPB = S // NCORES      # 256 tokens per core per batch
TPH = TPB // 2         # 128 tokens per core per half-batch (a2a chunk)
F32 = mybir.dt.float32
F16 = mybir.dt.float16
F8 = mybir.dt.float8e4
DR = mybir.MatmulPerfMode.DoubleRow
WSCALE = 1.0
EXP_SCALE = 0.125 / (WSCALE * WSCALE)
OUT_SCALE = 1.0


def build_nc():
    nc = bacc.Bacc(num_devices=NCORES)

    xt_d = nc.dram_tensor("xt", [D, T], F16, kind="ExternalInput")
    wq_d = nc.dram_tensor("wq", [D, FPC], F16, kind="ExternalInput")
    wk_d = nc.dram_tensor("wk", [D, FPC], F16, kind="ExternalInput")
    wv_d = nc.dram_tensor("wv", [D, FPC], F16, kind="ExternalInput")
    wo_d = nc.dram_tensor("wo", [D, D], F16, kind="ExternalInput")
    bo_d = nc.dram_tensor("bo", [1, D], F32, kind="ExternalInput")
    warm_in = nc.dram_tensor("warm_in", [NCORES, 16], F32, kind="Internal")
    warm_out = nc.dram_tensor("warm_out", [NCORES, 16], F32, kind="Internal")
    a2a_in = [nc.dram_tensor(f"a2a_in{i}", [D, TPH], F16, kind="Internal")
              for i in range(2 * B)]
    a2a_out = [nc.dram_tensor(f"a2a_out{i}", [D, TPH], F16, kind="Internal")
               for i in range(2 * B)]
    out_d = nc.dram_tensor("out", [B * TPB, D], F16, kind="ExternalOutput")

    groups = [list(range(NCORES))]

    with TileContext(nc) as tc:
        with (
            tc.tile_pool(name="const", bufs=1) as constp,
            tc.tile_pool(name="wts", bufs=1) as wp,
            tc.tile_pool(name="big", bufs=1) as bigp,
        ):
            # warmup collective first: absorbs the cc-channel setup
            # barrier while stage A runs.
            nc.gpsimd.collective_compute(
                "AllToAll", mybir.AluOpType.bypass, replica_groups=groups,
                ins=[warm_in[:, :]], outs=[warm_out[:, :]],
            )

            # --- weights / constants ---
            wq_sb = wp.tile([128, 8, FPC], F16)
            wk_sb = wp.tile([128, 8, FPC], F16)
            wv_sb = wp.tile([128, 8, FPC], F16)
            for w_sb, w_dram in ((wq_sb, wq_d), (wk_sb, wk_d), (wv_sb, wv_d)):
                for j in range(8):
                    nc.scalar.dma_start(w_sb[:, j, :], w_dram[j * 128:(j + 1) * 128, :])
            wo_sb = wp.tile([128, 8, D], F16)
            bo_row = constp.tile([1, D], F32)
            nc.scalar.dma_start(bo_row, bo_d[0:1, :])
            bo_bc = constp.tile([128, D], F32)
            nc.gpsimd.partition_broadcast(bo_bc, bo_row)

            # --- resident activations ---
            qt_sb = bigp.tile([128, T], F16)     # Q^T  [feat(2 heads x 64), tok]
            kt_sb = bigp.tile([128, T], F16)     # K^T
            ctxt = bigp.tile([128, T], F16)      # normalized ctx^T (fp16)
            v16 = bigp.tile([128, B, NKT, HPC, HDP], F16)  # [V_h|1] tiles
            ones_col = constp.tile([128, 1], F32)
            nc.gpsimd.memset(ones_col, 1.0)
            nc.vector.tensor_copy(
                v16[:, :, :, :, HD:HD + 1],
                ones_col[:, None, None, None, :].broadcast_to([128, B, NKT, HPC, 1]),
            )

            with (
                tc.tile_pool(name="xt", bufs=3) as xtp,
                tc.tile_pool(name="ex", bufs=6) as sbB,
                tc.tile_pool(name="nrm", bufs=4) as nrm,
                tc.tile_pool(name="ga", bufs=4) as gap,
                tc.tile_pool(name="sbO", bufs=4) as sbO,
                tc.tile_pool(name="psA", bufs=2, space="PSUM") as psA,
                tc.tile_pool(name="psS", bufs=2, space="PSUM") as psS,
                tc.tile_pool(name="psC", bufs=2, space="PSUM") as psC,
            ):
                APH = 2 * SPAN      # max stage-A phase width
                PHASES = [(0, 512), (512, 512), (1024, 1024),
                          (2048, 1024), (3072, 1024)]

                def emit_a_dma(ph, three_q=False):
                    t0, w = PHASES[ph]
                    xt = xtp.tile([128, 8, APH], F16, tag="xt")
                    for j in range(8):
                        if three_q:
                            eng = (nc.sync, nc.scalar, nc.gpsimd)[j % 3]
                        else:
                            eng = nc.sync if j % 2 == 0 else nc.scalar
                        eng.dma_start(
                            xt[:, j, :w],
                            xt_d[j * 128:(j + 1) * 128, t0:t0 + w])
                    return xt

                def emit_a_proj(xt, ph):
                    t0, w = PHASES[ph]
                    b = t0 // S
                    for hv in range(w // SPAN):
                        for w_sb, dst in ((wq_sb, qt_sb), (wk_sb, kt_sb)):
                            pp = psA.tile([128, SPAN], F32, tag="p")
                            for j in range(8):
                                nc.tensor.matmul(
                                    pp, w_sb[:, j, :],
                                    xt[:, j, hv * SPAN:(hv + 1) * SPAN],
                                    start=(j == 0), stop=(j == 7),
                                )
                            nc.vector.tensor_copy(
                                dst[:, t0 + hv * SPAN:t0 + (hv + 1) * SPAN], pp)
                    for t in range(w // 128):
                        kti = (t0 + t * 128 - b * S) // KT
                        pv = psA.tile([128, SPAN], F32, tag="p")
                        for j in range(8):
                            nc.tensor.matmul(
                                pv[:, 0:FPC],
                                xt[:, j, t * 128:(t + 1) * 128],
                                wv_sb[:, j, :],
                                start=(j == 0), stop=(j == 7),
                            )
                        for h in range(HPC):
                            nc.vector.tensor_copy(
                                v16[:, b, kti, h, 0:HD],
                                pv[:, h * HD:(h + 1) * HD],
                            )

                def emit_a(ph):
                    emit_a_proj(emit_a_dma(ph), ph)

                def emit_attn(b, qc):
                    q0 = b * S + qc * QC
                    for h in range(HPC):
                        pc = psC.tile([HDP, QC], F32, tag="c")
                        n_full = qc * 4
                        # full k-tiles below the diagonal, in DoubleRow pairs
                        for p in range(n_full // 2):
                            kt0 = 2 * p
                            ps = psS.tile([128, 2, QC], F32, tag="s")
                            for i in range(2):
                                nc.tensor.matmul(
                                    ps[:, i, :],
                                    kt_sb[h * HD:(h + 1) * HD,
                                          b * S + (kt0 + i) * KT:
                                          b * S + (kt0 + i + 1) * KT],
                                    qt_sb[h * HD:(h + 1) * HD, q0:q0 + QC],
                                    start=True, stop=True,
                                )
                            ex = sbB.tile([128, 2, QC], F16, tag="ex")
                            nc.scalar.activation(
                                ex, ps, mybir.ActivationFunctionType.Exp,
                                scale=EXP_SCALE,
                            )
                            for i in range(2):
                                nc.tensor.matmul(
                                    pc[:, :],
                                    v16[:, b, kt0 + i, h, :], ex[:, i, :],
                                    start=(p == 0 and i == 0), stop=False,
                                )
                        # 4 diagonal k-tiles, singles with causal zeroing
                        for dgi in range(4):
                            kt = qc * 4 + dgi
                            col_off = dgi * KT
                            n = QC - col_off
                            ps1 = psS.tile([128, 2, QC], F32, tag="s")
                            nc.tensor.matmul(
                                ps1[:, 0, :n],
                                kt_sb[h * HD:(h + 1) * HD,
                                      b * S + kt * KT:b * S + (kt + 1) * KT],
                                qt_sb[h * HD:(h + 1) * HD, q0 + col_off:q0 + QC],
                                start=True, stop=True,
                            )
                            ex1 = sbB.tile([128, 2, QC], F16, tag="ex")
                            nc.scalar.activation(
                                ex1[:, 0, :n], ps1[:, 0, :n],
                                mybir.ActivationFunctionType.Exp,
                                scale=EXP_SCALE,
                            )
                            nc.gpsimd.affine_select(
                                out=ex1[:, 0, 0:KT],
                                in_=ex1[:, 0, 0:KT],
                                compare_op=mybir.AluOpType.is_ge,
                                fill=0.0,
                                base=0,
                                pattern=[[1, KT]],
                                channel_multiplier=-1,
                            )
                            nc.tensor.matmul(
                                pc[:, col_off:QC],
                                v16[:, b, kt, h, :],
                                ex1[:, 0, :n],
                                start=(n_full == 0 and dgi == 0),
                                stop=(dgi == 3),
                            )
                        den = nrm.tile([1, QC], F32, tag="d")
                        nc.vector.tensor_copy(den, pc[HD:HD + 1, :])
                        rrow = nrm.tile([1, QC], F32, tag="r")
                        nc.vector.reciprocal_approx_fast(rrow, den)
                        rec64 = nrm.tile([HD, QC], F32, tag="b")
                        nc.gpsimd.partition_broadcast(rec64, rrow)
                        for pz in range(4):
                            z = pz * 128
                            nc.vector.tensor_mul(
                                ctxt[h * HD:(h + 1) * HD, q0 + z:q0 + z + 128],
                                pc[0:HD, z:z + 128], rec64[:, z:z + 128],
                            )

                def emit_a2a(b, hf):
                    i = 2 * b + hf
                    c0 = b * S + hf * (S // 2)
                    dma_eng = nc.scalar if i == 2 * B - 1 else nc.gpsimd
                    for d in range(NCORES):
                        dma_eng.dma_start(
                            a2a_in[i][d * 128:(d + 1) * 128, :],
                            ctxt[:, c0 + d * TPH:c0 + (d + 1) * TPH])
                    nc.gpsimd.collective_compute(
                        "AllToAll", mybir.AluOpType.bypass,
                        replica_groups=groups,
                        ins=[a2a_in[i][:, :]], outs=[a2a_out[i][:, :]],
                    )

                def emit_out(b, hf):
                    i = 2 * b + hf
                    ga = gap.tile([128, 8, TPH], F16, tag="ga")
                    for c in range(NCORES):
                        nc.sync.dma_start(
                            ga[:, c, :], a2a_out[i][c * 128:(c + 1) * 128, :])
                    so = sbO.tile([128, D], F16, tag="so")
                    for half in range(2):
                        po = psA.tile([128, SPAN], F32, tag="p")
                        for j in range(8):
                            nc.tensor.matmul(
                                po,
                                ga[:, j, :],
                                wo_sb[:, j, half * 512:(half + 1) * 512],
                                start=(j == 0), stop=(j == 7),
                            )
                        nc.vector.scalar_tensor_tensor(
                            so[:, half * 512:(half + 1) * 512],
                            po, OUT_SCALE, bo_bc[:, half * 512:(half + 1) * 512],
                            mybir.AluOpType.mult, mybir.AluOpType.add,
                        )
                    nc.gpsimd.dma_start(
                        out_d[b * TPB + hf * TPH:b * TPB + (hf + 1) * TPH, :], so)

                emit_a(0)
                emit_attn(0, 0)
                emit_a(1)
                emit_attn(0, 1)
                emit_a2a(0, 0)
                emit_a(2)
                emit_attn(0, 2)
                emit_attn(0, 3)
                emit_a2a(0, 1)
                emit_a(3)
                for j in range(8):
                    nc.scalar.dma_start(
                        wo_sb[:, j, :], wo_d[j * 128:(j + 1) * 128, :])
                emit_attn(1, 0)
                emit_attn(1, 1)
                emit_a(4)
                emit_a2a(1, 0)
                emit_out(0, 0)
                emit_attn(1, 2)
                emit_out(0, 1)
                emit_attn(1, 3)
                emit_out(1, 0)
                emit_a2a(1, 1)
                emit_out(1, 1)

    nc.finalize()
    return nc


_NC_CACHE = []


def make_in_maps(x, Wq, Wk, Wv, Wo, bo):
    x = np.asarray(x, dtype=np.float32).reshape(T, D)
    xt16 = np.ascontiguousarray(x.T).astype(np.float16)
    Wq = np.asarray(Wq, dtype=np.float32)
    Wk = np.asarray(Wk, dtype=np.float32)
    Wv = np.asarray(Wv, dtype=np.float32)
    wo16 = np.asarray(Wo, dtype=np.float32).astype(np.float16)
    bo = np.asarray(bo, dtype=np.float32).reshape(1, D)
    in_maps = []
    for c in range(NCORES):
        lo, hi = c * FPC, (c + 1) * FPC
        in_maps.append({
            "xt": xt16,
            "wq": np.ascontiguousarray(Wq[:, lo:hi]).astype(np.float16),
            "wk": np.ascontiguousarray(Wk[:, lo:hi]).astype(np.float16),
            "wv": np.ascontiguousarray(Wv[:, lo:hi]).astype(np.float16),
            "wo": wo16,
            "bo": bo,
        })
    return in_maps


def assemble_out(core_outs):
    # core r rows [b*256 + hf*128 + i] = batch b, s = hf*1024 + r*128 + i
    full = np.empty((B, S, D), dtype=np.float32)
    for r, o in enumerate(core_outs):
        o = np.asarray(o, dtype=np.float32)
        for b in range(B):
            for hf in range(2):
                full[b, hf * (S // 2) + r * TPH:hf * (S // 2) + (r + 1) * TPH] = \
                    o[b * TPB + hf * TPH:b * TPB + (hf + 1) * TPH]
    return full


def kernel(x, Wq, Wk, Wv, Wo, bo):
    if not _NC_CACHE:
        _NC_CACHE.append(build_nc())
    nc = _NC_CACHE[0]
    in_maps = make_in_maps(x, Wq, Wk, Wv, Wo, bo)
    res = run_bass_kernel_spmd(nc, in_maps, core_ids=list(range(NCORES)))
    return assemble_out([r["out"] for r in res.results])


# revision 34
# speedup vs baseline: 1.1656x; 1.0228x over previous
"""Causal multi-head attention on 8 TRN2 NeuronCores.

Sharding: tensor-parallel over heads. Each core owns 2 of the 16 heads:
column slices of Wq/Wk/Wv. The output projection is fully local: after
attention, small half-batch AllToAlls (512KB fp16 each) redistribute
ctx^T so every core holds ALL 1024 features for its 128-token shard of
each half-batch, then out = ctx @ Wo + bo locally -- no reduction
collective at all (vs. row-parallel Wo + ReduceScatter, this moves 8x
less data). A tiny warmup AllToAll is issued first so the one-time cc
channel-setup barrier overlaps stage A instead of delaying the first
real collective.

Shapes (hardcoded): B=2, S=2048, D=1024, H=16, HD=64.

Numerics: all-fp16 operands, fp32 PSUM accumulation and softmax
denominators (fp8 was tested and rejected: a random-sign dot product
keeps per-element relative error, so fp8 anywhere on the Q/K/V/x path
costs 1e-2..5e-2 of output error).

x is pre-transposed on the host (free) and DMA'd linearly -- no
on-device transpose at all. Stage-A phases are sized progressively
(512, 512, 1024, 1024, 1024 tokens) so the PE starts projecting ~5us
in while the rest of x^T streams. DMA queue roles are fixed: x^T loads
on sync+scalar, dependency-gated DMAs (a2a_in, gather, out) on gpsimd
so a waiting DMA never head-of-line-blocks a prefetch.

Per-core dataflow:
  A) per phase: QT/KT = W_c.T @ xT (8-step K=1024 accumulation,
     N=512); V in natural [tok, feat] layout via lhsT=xT tiles, stored
     per (batch, k-tile, head) as [128, 65] = [V_head | ones-column].
  B) per (batch, 512-query-chunk, head): scores^T[k,q] = KT.T @ QT
     (K=64), exp on ACT straight out of PSUM (scale=0.125, no max
     subtraction: |scores|/8 <~ 3) into paired 2-bank tiles -> fp16,
     causal zeroing of diagonal blocks via gpsimd affine_select on the
     exp output (keep col >= row), then ctx^T[d,q] accumulated over
     k-tiles with lhsT=[V|1] so PSUM row 64 is the softmax denominator.
     Normalize via DVE reciprocal_approx_fast (5x faster than the
     table-based reciprocal, 18-bit accuracy is plenty for softmax
     denominators; input staged through SBUF -- the custom-DVE op
     NaNs on PSUM reads) + gpsimd partition-broadcast + DVE multiply
     in 128-column pieces (lets each a2a_in chunk DMA fly as soon as
     its columns are normalized).
  C) per half-batch: 8 DMAs push ctx^T [128, 128]-token chunks to
     DRAM, AllToAll redistributes, gather to SBUF, local out-proj
     (K=1024 vs full Wo), bias via DVE scalar_tensor_tensor, DMA out.
     Out-projections are emitted late in the PE stream so a straggling
     collective (cross-core start skew is 20-50us) never stalls the PE
     mid-kernel; Wo's 2MB load is deferred past the x^T prefetch window.

The final half-batch's a2a_in DMAs issue from the scalar queue
(idle after the last exp) instead of gpsimd, shortening the tail
dispatch chain.

Measured: 250-265us HW exec typical, +/-15us cross-core-skew noise
(baseline 423us); rel err 5.4e-4 (max), 5.7e-4 (l2).
"""

import numpy as np

import concourse.bacc as bacc
import concourse.bass as bass
import concourse.mybir as mybir
from concourse.bass_utils import run_bass_kernel_spmd
from concourse.tile import TileContext

B, S, D, H = 2, 2048, 1024, 16
HD = D // H            # 64
NCORES = 8
HPC = H // NCORES      # 2 heads per core
FPC = HPC * HD         # 128 feature cols per core
T = B * S              # 4096 tokens
SPAN = 512             # stage-A token span
NSPAN = T // SPAN      # 8
QC = 512               # query chunk
NCHB = S // QC         # 4 chunks per batch
KT = 128               # k-tile size
HDP = HD + 1           # [V|1] tile width
NKT = S // KT          # 16 k-tiles per batch
TPB = S // NCORES      # 256 tokens per core per batch
TPH = TPB // 2         # 128 tokens per core per half-batch (a2a chunk)
F32 = mybir.dt.float32
F16 = mybir.dt.float16
F8 = mybir.dt.float8e4
DR = mybir.MatmulPerfMode.DoubleRow
WSCALE = 1.0
EXP_SCALE = 0.125 / (WSCALE * WSCALE)
OUT_SCALE = 1.0


def build_nc():
    nc = bacc.Bacc(num_devices=NCORES)

    xt_d = nc.dram_tensor("xt", [D, T], F16, kind="ExternalInput")
    wq_d = nc.dram_tensor("wq", [D, FPC], F16, kind="ExternalInput")
    wk_d = nc.dram_tensor("wk", [D, FPC], F16, kind="ExternalInput")
    wv_d = nc.dram_tensor("wv", [D, FPC], F16, kind="ExternalInput")
    wo_d = nc.dram_tensor("wo", [D, D], F16, kind="ExternalInput")
    bo_d = nc.dram_tensor("bo", [1, D], F32, kind="ExternalInput")
    warm_in = nc.dram_tensor("warm_in", [NCORES, 16], F32, kind="Internal")
    warm_out = nc.dram_tensor("warm_out", [NCORES, 16], F32, kind="Internal")
    a2a_in = [nc.dram_tensor(f"a2a_in{i}", [D, TPH], F16, kind="Internal")
              for i in range(2 * B)]
    a2a_out = [nc.dram_tensor(f"a2a_out{i}", [D, TPH], F16, kind="Internal")
               for i in range(2 * B)]
    out_d = nc.dram_tensor("out", [B * TPB, D], F16, kind="ExternalOutput")

    groups = [list(range(NCORES))]

    with TileContext(nc) as tc:
        with (
            tc.tile_pool(name="const", bufs=1) as constp,
            tc.tile_pool(name="wts", bufs=1) as wp,
            tc.tile_pool(name="big", bufs=1) as bigp,
        ):
            # warmup collective first: absorbs the cc-channel setup
            # barrier while stage A runs.
            nc.gpsimd.collective_compute(
                "AllToAll", mybir.AluOpType.bypass, replica_groups=groups,
                ins=[warm_in[:, :]], outs=[warm_out[:, :]],
            )

            # --- weights / constants ---
            wq_sb = wp.tile([128, 8, FPC], F16)
            wk_sb = wp.tile([128, 8, FPC], F16)
            wv_sb = wp.tile([128, 8, FPC], F16)
            for w_sb, w_dram in ((wq_sb, wq_d), (wk_sb, wk_d), (wv_sb, wv_d)):
                for j in range(8):
                    nc.scalar.dma_start(w_sb[:, j, :], w_dram[j * 128:(j + 1) * 128, :])
            wo_sb = wp.tile([128, 8, D], F16)
            bo_row = constp.tile([1, D], F32)
            nc.scalar.dma_start(bo_row, bo_d[0:1, :])
            bo_bc = constp.tile([128, D], F32)
            nc.gpsimd.partition_broadcast(bo_bc, bo_row)

            # --- resident activations ---
            qt_sb = bigp.tile([128, T], F16)     # Q^T  [feat(2 heads x 64), tok]
            kt_sb = bigp.tile([128, T], F16)     # K^T
            ctxt = bigp.tile([128, T], F16)      # normalized ctx^T (fp16)
            v16 = bigp.tile([128, B, NKT, HPC, HDP], F16)  # [V_h|1] tiles
            ones_col = constp.tile([128, 1], F32)
            nc.gpsimd.memset(ones_col, 1.0)
            nc.vector.tensor_copy(
                v16[:, :, :, :, HD:HD + 1],
                ones_col[:, None, None, None, :].broadcast_to([128, B, NKT, HPC, 1]),
            )

            with (
                tc.tile_pool(name="xt", bufs=3) as xtp,
                tc.tile_pool(name="ex", bufs=6) as sbB,
                tc.tile_pool(name="nrm", bufs=4) as nrm,
                tc.tile_pool(name="ga", bufs=4) as gap,
                tc.tile_pool(name="sbO", bufs=4) as sbO,
                tc.tile_pool(name="psA", bufs=2, space="PSUM") as psA,
                tc.tile_pool(name="psS", bufs=2, space="PSUM") as psS,
                tc.tile_pool(name="psC", bufs=2, space="PSUM") as psC,
            ):
                APH = 2 * SPAN      # max stage-A phase width
                PHASES = [(0, 512), (512, 512), (1024, 1024),
                          (2048, 1024), (3072, 1024)]

                def emit_a_dma(ph, three_q=False):
                    t0, w = PHASES[ph]
                    xt = xtp.tile([128, 8, APH], F16, tag="xt")
                    for j in range(8):
                        if three_q:
                            eng = (nc.sync, nc.scalar, nc.gpsimd)[j % 3]
                        else:
                            eng = nc.sync if j % 2 == 0 else nc.scalar
                        eng.dma_start(
                            xt[:, j, :w],
                            xt_d[j * 128:(j + 1) * 128, t0:t0 + w])
                    return xt

                def emit_a_proj(xt, ph):
                    t0, w = PHASES[ph]
                    b = t0 // S
                    for hv in range(w // SPAN):
                        for w_sb, dst in ((wq_sb, qt_sb), (wk_sb, kt_sb)):
                            pp = psA.tile([128, SPAN], F32, tag="p")
                            for j in range(8):
                                nc.tensor.matmul(
                                    pp, w_sb[:, j, :],
                                    xt[:, j, hv * SPAN:(hv + 1) * SPAN],
                                    start=(j == 0), stop=(j == 7),
                                )
                            nc.vector.tensor_copy(
                                dst[:, t0 + hv * SPAN:t0 + (hv + 1) * SPAN], pp)
                    for t in range(w // 128):
                        kti = (t0 + t * 128 - b * S) // KT
                        pv = psA.tile([128, SPAN], F32, tag="p")
                        for j in range(8):
                            nc.tensor.matmul(
                                pv[:, 0:FPC],
                                xt[:, j, t * 128:(t + 1) * 128],
                                wv_sb[:, j, :],
                                start=(j == 0), stop=(j == 7),
                            )
                        for h in range(HPC):
                            nc.vector.tensor_copy(
                                v16[:, b, kti, h, 0:HD],
                                pv[:, h * HD:(h + 1) * HD],
                            )

                def emit_a(ph):
                    emit_a_proj(emit_a_dma(ph), ph)

                def emit_attn(b, qc):
                    q0 = b * S + qc * QC
                    for h in range(HPC):
                        pc = psC.tile([HDP, QC], F32, tag="c")
                        n_full = qc * 4
                        # full k-tiles below the diagonal, in DoubleRow pairs
                        for p in range(n_full // 2):
                            kt0 = 2 * p
                            ps = psS.tile([128, 2, QC], F32, tag="s")
                            for i in range(2):
                                nc.tensor.matmul(
                                    ps[:, i, :],
                                    kt_sb[h * HD:(h + 1) * HD,
                                          b * S + (kt0 + i) * KT:
                                          b * S + (kt0 + i + 1) * KT],
                                    qt_sb[h * HD:(h + 1) * HD, q0:q0 + QC],
                                    start=True, stop=True,
                                )
                            ex = sbB.tile([128, 2, QC], F16, tag="ex")
                            nc.scalar.activation(
                                ex, ps, mybir.ActivationFunctionType.Exp,
                                scale=EXP_SCALE,
                            )
                            for i in range(2):
                                nc.tensor.matmul(
                                    pc[:, :],
                                    v16[:, b, kt0 + i, h, :], ex[:, i, :],
                                    start=(p == 0 and i == 0), stop=False,
                                )
                        # 4 diagonal k-tiles, singles with causal zeroing
                        for dgi in range(4):
                            kt = qc * 4 + dgi
                            col_off = dgi * KT
                            n = QC - col_off
                            ps1 = psS.tile([128, 2, QC], F32, tag="s")
                            nc.tensor.matmul(
                                ps1[:, 0, :n],
                                kt_sb[h * HD:(h + 1) * HD,
                                      b * S + kt * KT:b * S + (kt + 1) * KT],
                                qt_sb[h * HD:(h + 1) * HD, q0 + col_off:q0 + QC],
                                start=True, stop=True,
                            )
                            ex1 = sbB.tile([128, 2, QC], F16, tag="ex")
                            nc.scalar.activation(
                                ex1[:, 0, :n], ps1[:, 0, :n],
                                mybir.ActivationFunctionType.Exp,
                                scale=EXP_SCALE,
                            )
                            nc.gpsimd.affine_select(
                                out=ex1[:, 0, 0:KT],
                                in_=ex1[:, 0, 0:KT],
                                compare_op=mybir.AluOpType.is_ge,
                                fill=0.0,
                                base=0,
                                pattern=[[1, KT]],
                                channel_multiplier=-1,
                            )
                            nc.tensor.matmul(
                                pc[:, col_off:QC],
                                v16[:, b, kt, h, :],
                                ex1[:, 0, :n],
                                start=(n_full == 0 and dgi == 0),
                                stop=(dgi == 3),
                            )
                        den = nrm.tile([1, QC], F32, tag="d")
                        nc.vector.tensor_copy(den, pc[HD:HD + 1, :])
                        rrow = nrm.tile([1, QC], F32, tag="r")
                        nc.vector.reciprocal_approx_fast(rrow, den)
                        rec64 = nrm.tile([HD, QC], F32, tag="b")
                        nc.gpsimd.partition_broadcast(rec64, rrow)
                        for pz in range(4):
                            z = pz * 128
                            nc.vector.tensor_mul(
                                ctxt[h * HD:(h + 1) * HD, q0 + z:q0 + z + 128],
                                pc[0:HD, z:z + 128], rec64[:, z:z + 128],
                            )

                def emit_a2a(b, hf):
                    i = 2 * b + hf
                    c0 = b * S + hf * (S // 2)
                    last = i == 2 * B - 1
                    for d in range(NCORES):
                        dma_eng = (nc.scalar if d % 2 == 0 else nc.sync) \
                            if last else nc.gpsimd
                        dma_eng.dma_start(
                            a2a_in[i][d * 128:(d + 1) * 128, :],
                            ctxt[:, c0 + d * TPH:c0 + (d + 1) * TPH])
                    nc.gpsimd.collective_compute(
                        "AllToAll", mybir.AluOpType.bypass,
                        replica_groups=groups,
                        ins=[a2a_in[i][:, :]], outs=[a2a_out[i][:, :]],
                    )

                def emit_out(b, hf):
                    i = 2 * b + hf
                    ga = gap.tile([128, 8, TPH], F16, tag="ga")
                    for c in range(NCORES):
                        eng = nc.sync if c % 2 == 0 else nc.scalar
                        eng.dma_start(
                            ga[:, c, :], a2a_out[i][c * 128:(c + 1) * 128, :])
                    so = sbO.tile([128, D], F16, tag="so")
                    for half in range(2):
                        po = psA.tile([128, SPAN], F32, tag="p")
                        for j in range(8):
                            nc.tensor.matmul(
                                po,
                                ga[:, j, :],
                                wo_sb[:, j, half * 512:(half + 1) * 512],
                                start=(j == 0), stop=(j == 7),
                            )
                        nc.vector.scalar_tensor_tensor(
                            so[:, half * 512:(half + 1) * 512],
                            po, OUT_SCALE, bo_bc[:, half * 512:(half + 1) * 512],
                            mybir.AluOpType.mult, mybir.AluOpType.add,
                        )
                    nc.gpsimd.dma_start(
                        out_d[b * TPB + hf * TPH:b * TPB + (hf + 1) * TPH, :], so)

                emit_a(0)
                emit_attn(0, 0)
                emit_a(1)
                emit_attn(0, 1)
                emit_a2a(0, 0)
                emit_a(2)
                emit_attn(0, 2)
                emit_attn(0, 3)
                emit_a2a(0, 1)
                emit_a(3)
                for j in range(8):
                    nc.scalar.dma_start(
                        wo_sb[:, j, :], wo_d[j * 128:(j + 1) * 128, :])
                emit_attn(1, 0)
                emit_attn(1, 1)
                emit_a(4)
                emit_a2a(1, 0)
                emit_out(0, 0)
                emit_attn(1, 2)
                emit_out(0, 1)
                emit_attn(1, 3)
                emit_out(1, 0)
                emit_a2a(1, 1)
                emit_out(1, 1)

    nc.finalize()
    return nc


_NC_CACHE = []


def make_in_maps(x, Wq, Wk, Wv, Wo, bo):
    x = np.asarray(x, dtype=np.float32).reshape(T, D)
    xt16 = np.ascontiguousarray(x.T).astype(np.float16)
    Wq = np.asarray(Wq, dtype=np.float32)
    Wk = np.asarray(Wk, dtype=np.float32)
    Wv = np.asarray(Wv, dtype=np.float32)
    wo16 = np.asarray(Wo, dtype=np.float32).astype(np.float16)
    bo = np.asarray(bo, dtype=np.float32).reshape(1, D)
    in_maps = []
    for c in range(NCORES):
        lo, hi = c * FPC, (c + 1) * FPC
        in_maps.append({
            "xt": xt16,
            "wq": np.ascontiguousarray(Wq[:, lo:hi]).astype(np.float16),
            "wk": np.ascontiguousarray(Wk[:, lo:hi]).astype(np.float16),
            "wv": np.ascontiguousarray(Wv[:, lo:hi]).astype(np.float16),
            "wo": wo16,
            "bo": bo,
        })
    return in_maps


def assemble_out(core_outs):
    # core r rows [b*256 + hf*128 + i] = batch b, s = hf*1024 + r*128 + i
    full = np.empty((B, S, D), dtype=np.float32)
    for r, o in enumerate(core_outs):
        o = np.asarray(o, dtype=np.float32)
        for b in range(B):
            for hf in range(2):
                full[b, hf * (S // 2) + r * TPH:hf * (S // 2) + (r + 1) * TPH] = \
                    o[b * TPB + hf * TPH:b * TPB + (hf + 1) * TPH]
    return full


def kernel(x, Wq, Wk, Wv, Wo, bo):
    if not _NC_CACHE:
        _NC_CACHE.append(build_nc())
    nc = _NC_CACHE[0]
    in_maps = make_in_maps(x, Wq, Wk, Wv, Wo, bo)
    res = run_bass_kernel_spmd(nc, in_maps, core_ids=list(range(NCORES)))
    return assemble_out([r["out"] for r in res.results])
